# revision 13
# baseline (speedup 1.0000x reference)
"""Trainium2 Bass kernel for the inverse deep-hough-transform gather-reduce.

out[n, c, y, x] = sum_k acc[n, c, k, rho_idx[k, y, x]]

Design (v7): hybrid 2-D-tile one-hot matmul gather, fp8-e3m4 data + one-hots
----------------------------------------------------------------------------
The rho index r(k,y,x) = round(cos_k*xc + sin_k*yc) drifts by |cos| per
x-step and |sin| per y-step.  Angles split into two families:

- rowpart (|sin| >= |cos|): output tiles of 4 y x 32 x; the rho window over
  an (8-row halfblock x 32-col quarter) is |cos|*32 + |sin|*8 + O(1) wide.
- colpart (|cos| > |sin|): transposed, tiles of 4 x x 32 y.

Windows pack (FFD) into 128-row contraction bins; one bin = one PE matmul
per 128-pixel tile instance: out[128 px, 512 nc] += OH.T @ slab.  Both OH
(0/1 one-hot) and slab data are fp8 e3m4: 1.0 is exact, the PE handles
e3m4 subnormals exactly (probed), and single-pass e3m4 quantization of the
N(0,1) accumulator costs 1.34% end-to-end vs the 2e-2 gate.  fp8 halves
slab+weight DMA vs bf16, which is what affords the narrow g=8 windows
(B=~16 bins -> 512 matmuls/core at ~218ns = the PE floor).

Sharding: core c owns output rows [16c,16c+16) for rowpart and cols
[16c,16c+16) for colpart -- 2 families x 4 quarters x 4 tile-quads = 32
instances of B matmuls; 16 slab groups (fam x quarter x halfblock) all
resident in SBUF, loaded once.  Outputs evict as bf16; the full result is
host-assembled as out[y,x] = rowpart_{y//16}[y%16, x] + colpart_{x//16}
[x%16, y].  The SPMD instruction stream is identical on every core.
"""

from contextlib import ExitStack

import ml_dtypes
import numpy as np

import concourse.bass as bass
from concourse import mybir
from concourse.bass_utils import run_bass_kernel_spmd

BF16 = ml_dtypes.bfloat16
E3 = ml_dtypes.float8_e3m4

# Problem constants (hardcoded per the harness contract).
N, C, A, R = 4, 128, 180, 184
H = W = 128
NC = N * C  # 512
NCORES = 8
TY, TX = 4, 32  # tile shape (iterate-dim extent x gather-dim extent)
NQ = 128 // TX  # quarters
NG = 16 // TY  # tile quads per (core, quarter)
GH = 8  # halfblock rows (slab window granularity along the iterate dim)
NGRP = 2 * NQ * 2  # slab groups: fam x quarter x halfblock
NBANK = 8  # PSUM banks
NWRING = 4  # weight ring depth (instances)
NOBUF = 8  # output staging buffers

_cache = {}


def _rho_table():
    """r[k, y, x] int32 rho index; always in [0, R) for this geometry."""
    if "r" not in _cache:
        k = np.arange(A)
        theta = k * (np.pi / A)
        cos_t, sin_t = np.cos(theta), np.sin(theta)
        y, x = np.meshgrid(np.arange(H), np.arange(W), indexing="ij")
        xc = (x - W // 2).astype(np.float64)
        yc = (y - H // 2).astype(np.float64)
        r = np.round(cos_t[:, None, None] * xc[None] + sin_t[:, None, None] * yc[None])
        r = r.astype(np.int64) + R // 2
        assert (r >= 0).all() and (r < R).all()
        _cache["r"] = r.astype(np.int32)
        _cache["fam_row"] = np.abs(sin_t) >= np.abs(cos_t)
    return _cache["r"]


def _blk(r, f, c, q, h, k):
    """The (halfblock x quarter) index block for angle k, family f."""
    s = 16 * c + GH * h
    if f == 0:
        return r[k, s : s + GH, TX * q : TX * q + TX]
    return r[k, TX * q : TX * q + TX, s : s + GH]


def _geometry():
    """Families, global lane widths, FFD bin layout (SPMD-uniform)."""
    if "geo" in _cache:
        return _cache["geo"]
    r = _rho_table()
    fam_row = _cache["fam_row"]

    width = {}
    for k in range(A):
        f = 0 if fam_row[k] else 1
        ws = [
            int(_blk(r, f, c, q, h, k).max() - _blk(r, f, c, q, h, k).min()) + 1
            for c in range(NCORES)
            for q in range(NQ)
            for h in range(2)
        ]
        width[k] = max(ws)
        assert width[k] <= 128

    def ffd(items):
        bins = []
        for w_, kk in sorted(items, reverse=True):
            for b in bins:
                if b[0] + w_ <= 128:
                    b[0] += w_
                    b[1].append((kk, w_))
                    break
            else:
                bins.append([w_, [(kk, w_)]])
        return [b[1] for b in bins]

    fams = []
    for f in range(2):
        ks = [k for k in range(A) if fam_row[k] == (f == 0)]
        fams.append(ffd([(width[k], k) for k in ks]))
    B = max(len(fams[0]), len(fams[1]))
    bins = [[], []]
    for f in range(2):
        for lane_list in fams[f]:
            out, base = [], 0
            for k, w_ in lane_list:
                out.append((k, w_, base))
                base += w_
            bins[f].append(out)
        while len(bins[f]) < B:
            bins[f].append([])

    _cache["geo"] = dict(bins=bins, B=B)
    return _cache["geo"]


def _host_weights():
    """Per-core one-hot tables [32, 128, B*128] e3m4 and slab offsets."""
    if "wts" in _cache:
        return _cache["wts"], _cache["offs"]
    geo = _geometry()
    r = _rho_table()
    B = geo["B"]
    bins = geo["bins"]

    wts = []
    offs = []  # offs[c][(f,q,h,bi)] = [(k, w, base, o)]
    for c in range(NCORES):
        w_tab = np.zeros((32, 128, B * 128), E3)
        omap = {}
        for f in range(2):
            for q in range(NQ):
                for h in range(2):
                    for bi, lanes in enumerate(bins[f]):
                        entry = []
                        for k, wd, base in lanes:
                            blk = _blk(r, f, c, q, h, k)
                            lo, hi = int(blk.min()), int(blk.max())
                            o = min(lo, R - wd)
                            assert 0 <= o and o + wd > hi
                            entry.append((k, wd, base, o))
                        omap[(f, q, h, bi)] = entry
                for g in range(NG):
                    i = f * 16 + q * NG + g
                    h = g // 2
                    for bi in range(B):
                        for k, wd, base, o in omap[(f, q, h, bi)]:
                            blk = _blk(r, f, c, q, h, k)
                            loc = TY * (g % 2)
                            # sub[it, gt]: iterate-dim-major tile indices
                            sub = (
                                blk[loc : loc + TY, :]
                                if f == 0
                                else blk[:, loc : loc + TY].T
                            )
                            rows = (sub - o + base).ravel()
                            w_tab[i, rows, bi * 128 + np.arange(128)] = 1
        wts.append(w_tab)
        offs.append(omap)
    _cache["wts"] = wts
    _cache["offs"] = offs
    return wts, offs


def _build_nc():
    if "nc" in _cache:
        return _cache["nc"]
    geo = _geometry()
    B = geo["B"]

    nc = bass.Bass("TRN2", debug=False, target_bir_lowering=False, num_devices=NCORES)
    wts_d = nc.dram_tensor(
        "wts", [32, 128, B * 128], mybir.dt.float8e3, kind="ExternalInput"
    ).ap()
    slab_d = nc.dram_tensor(
        "slabs", [NGRP, 128, B * NC], mybir.dt.float8e3, kind="ExternalInput"
    ).ap()
    outr_d = nc.dram_tensor(
        "outr", [16, 128, NC], mybir.dt.bfloat16, kind="ExternalOutput"
    ).ap()
    outc_d = nc.dram_tensor(
        "outc", [16, 128, NC], mybir.dt.bfloat16, kind="ExternalOutput"
    ).ap()

    ctx = ExitStack()
    _cache["ctx"] = ctx
    slabs_sb = ctx.enter_context(
        nc.sbuf_tensor("slabs_sb", [128, NGRP * B * NC], mybir.dt.float8e3)
    )
    wring = ctx.enter_context(
        nc.sbuf_tensor("wring", [128, NWRING * B * 128], mybir.dt.float8e3)
    )
    obuf = ctx.enter_context(
        nc.sbuf_tensor("obuf", [128, NOBUF * NC], mybir.dt.bfloat16)
    )
    ps = [
        ctx.enter_context(nc.psum_tensor(f"ps{i}", [128, NC], mybir.dt.float32))
        for i in range(NBANK)
    ]
    mm_sem = ctx.enter_context(nc.semaphore("mm_sem"))
    cp_sem = ctx.enter_context(nc.semaphore("cp_sem"))
    dump_sems = [
        ctx.enter_context(nc.semaphore(f"dump{s}")) for s in range(NOBUF)
    ]
    wt_sems = [ctx.enter_context(nc.semaphore(f"wt{s}")) for s in range(NWRING)]
    sl_sems = [ctx.enter_context(nc.semaphore(f"sl{g}")) for g in range(NGRP)]
    sl0b_sem = ctx.enter_context(nc.semaphore("sl0b"))
    block = ctx.enter_context(nc.Block(no_gpsimd_drain=True))

    HB = B // 2

    def _slab_dma(eng, grp):
        col = grp * B * NC
        if grp == 0:
            # chunked for a fast PE start: first HB bins, then the rest
            eng.dma_start(
                slabs_sb[:, col : col + HB * NC], slab_d[0, :, : HB * NC]
            ).then_inc(sl_sems[0], 16)
            eng.dma_start(
                slabs_sb[:, col + HB * NC : col + B * NC],
                slab_d[0, :, HB * NC : B * NC],
            ).then_inc(sl0b_sem, 16)
        else:
            eng.dma_start(
                slabs_sb[:, col : col + B * NC], slab_d[grp]
            ).then_inc(sl_sems[grp], 16)

    @block.gpsimd
    def _(gpsimd):
        for i in range(32):
            if i >= NWRING:
                gpsimd.wait_ge(mm_sem, i - NWRING + 1)
            base = (i % NWRING) * B * 128
            gpsimd.dma_start(
                wring[:, base : base + B * 128], wts_d[i]
            ).then_inc(wt_sems[i % NWRING], 16)

    @block.scalar
    def _(scalar):
        # family-0 slab groups stream up front in consumption order;
        # family-1 groups are issued once the pipeline is running.
        for grp in range(8):
            _slab_dma(scalar, grp)
        for i in range(32):
            scalar.wait_ge(mm_sem, i + 1)
            if i >= NOBUF:
                scalar.wait_ge(dump_sems[i % NOBUF], 16 * TY * (i // NOBUF))
            col = (i % NOBUF) * NC
            scalar.copy(obuf[:, col : col + NC], ps[i % NBANK][:]).then_inc(cp_sem, 1)
            if i == 2:
                for grp in range(8, 12):
                    _slab_dma(scalar, grp)
            if i == 8:
                for grp in range(12, 16):
                    _slab_dma(scalar, grp)

    @block.tensor
    def _(tensor):
        # Warm the PE clock gate during the DMA prologue with junk matmuls
        # (bank 7 is cleared by instance 7's start=True).
        wq = (NWRING - 1) * B * 128
        for _ in range(48):
            tensor.matmul(
                out=ps[NBANK - 1][:, :128],
                lhsT=wring[:, wq : wq + 128],
                rhs=wring[:, wq : wq + 128],
                start=True,
                stop=True,
                skip_group_check=True,
            )
        for i in range(32):
            f, rem = i // 16, i % 16
            q, g = rem // NG, rem % NG
            grp = f * 8 + q * 2 + g // 2
            if i >= NBANK:
                tensor.wait_ge(cp_sem, i - NBANK + 1)
            tensor.wait_ge(wt_sems[i % NWRING], 16 * (i // NWRING + 1))
            if g % 2 == 0:
                tensor.wait_ge(sl_sems[grp], 16)
            sl0b_wait = i == 0
            wbase = (i % NWRING) * B * 128
            scol = grp * B * NC
            for b in range(B):
                if sl0b_wait and b == HB:
                    tensor.wait_ge(sl0b_sem, 16)
                mm = tensor.matmul(
                    out=ps[i % NBANK][:],
                    lhsT=wring[:, wbase + b * 128 : wbase + (b + 1) * 128],
                    rhs=slabs_sb[:, scol + b * NC : scol + (b + 1) * NC],
                    start=(b == 0),
                    stop=(b == B - 1),
                )
            mm.then_inc(mm_sem, 1)

    @block.sync
    def _(sync):
        for i in range(32):
            sync.wait_ge(cp_sem, i + 1)
            f, rem = i // 16, i % 16
            q, g = rem // NG, rem % NG
            out_d = outr_d if f == 0 else outc_d
            col = (i % NOBUF) * NC
            for t in range(TY):
                sync.dma_start(
                    out_d[TY * g + t, TX * q : TX * q + TX],
                    obuf[t * TX : (t + 1) * TX, col : col + NC],
                ).then_inc(dump_sems[i % NOBUF], 16)

    _cache["nc"] = nc
    return nc


def _install_ntff_hook():
    """Provide the antenv.axon_hooks shim the image lacks, wiring the
    ctypes NTFF profiler from trn_agent_boot."""
    import sys
    import types

    if "antenv.axon_hooks" in sys.modules:
        return
    import antenv
    from trn_agent_boot.trn_boot import _ntff_profile_via_ctypes

    mod = types.ModuleType("antenv.axon_hooks")
    hook = _ntff_profile_via_ctypes("/opt/axon/libaxon_pjrt.so")
    mod.get_axon_ntff_profile_hook = lambda: hook
    mod.set_axon_ntff_profile_hook = lambda h: None
    sys.modules["antenv.axon_hooks"] = mod
    antenv.axon_hooks = mod


def hw_exec_time_ns(trace_cores=None):
    """Re-run the last kernel() invocation with tracing; return max core ns."""
    _install_ntff_hook()
    nc = _cache["nc"]
    res = run_bass_kernel_spmd(
        nc,
        _cache["in_maps"],
        core_ids=list(range(NCORES)),
        trace=True,
        trace_cores=trace_cores,
    )
    _cache["trace"] = res
    return res.exec_time_ns


def kernel(accumulator, out_H=128, out_W=128, numangle=180, numrho=184):
    accumulator = np.asarray(accumulator, np.float32)
    assert accumulator.shape == (N, C, A, R), accumulator.shape
    assert int(out_H) == H and int(out_W) == W
    assert int(numangle) == A and int(numrho) == R

    geo = _geometry()
    B = geo["B"]
    wts, offs = _host_weights()
    nc = _build_nc()

    # acc_t[k, rho, nc] e3m4 - slab source (single rounding from f32).
    acc_t = np.ascontiguousarray(
        accumulator.reshape(NC, A, R).transpose(1, 2, 0)
    ).astype(E3)

    in_maps = []
    for c in range(NCORES):
        slabs = np.zeros((NGRP, 128, B, NC), E3)
        for f in range(2):
            for q in range(NQ):
                for h in range(2):
                    grp = f * 8 + q * 2 + h
                    for bi in range(B):
                        for k, wd, base, o in offs[c][(f, q, h, bi)]:
                            slabs[grp, base : base + wd, bi] = acc_t[k, o : o + wd]
        in_maps.append({"wts": wts[c], "slabs": slabs.reshape(NGRP, 128, B * NC)})
    _cache["in_maps"] = in_maps
    res = run_bass_kernel_spmd(nc, in_maps, core_ids=list(range(NCORES)))

    # Unshard: out[y, x, nc] = rowpart[y] + colpart[x] (transposed).
    total = np.zeros((H, W, NC), np.float64)
    for c in range(NCORES):
        total[16 * c : 16 * c + 16] += res.results[c]["outr"].astype(np.float64)
        total[:, 16 * c : 16 * c + 16] += (
            res.results[c]["outc"].astype(np.float64).transpose(1, 0, 2)
        )
    return total.transpose(2, 0, 1).reshape(N, C, H, W).astype(np.float32)


# revision 14
# speedup vs baseline: 1.1668x; 1.1668x over previous
"""Trainium2 Bass kernel for the inverse deep-hough-transform gather-reduce.

out[n, c, y, x] = sum_k acc[n, c, k, rho_idx[k, y, x]]

Design (v7): hybrid 2-D-tile one-hot matmul gather, fp8-e3m4 data + one-hots
----------------------------------------------------------------------------
The rho index r(k,y,x) = round(cos_k*xc + sin_k*yc) drifts by |cos| per
x-step and |sin| per y-step.  Angles split into two families:

- rowpart (|sin| >= |cos|): output tiles of 4 y x 32 x; the rho window over
  an (8-row halfblock x 32-col quarter) is |cos|*32 + |sin|*8 + O(1) wide.
- colpart (|cos| > |sin|): transposed, tiles of 4 x x 32 y.

Windows pack (FFD) into 128-row contraction bins; one bin = one PE matmul
per 128-pixel tile instance: out[128 px, 512 nc] += OH.T @ slab.  Both OH
(0/1 one-hot) and slab data are fp8 e3m4: 1.0 is exact, the PE handles
e3m4 subnormals exactly (probed), and single-pass e3m4 quantization of the
N(0,1) accumulator costs 1.34% end-to-end vs the 2e-2 gate.  fp8 halves
slab+weight DMA vs bf16, which is what affords the narrow g=8 windows
(B=~16 bins -> 512 matmuls/core at ~218ns = the PE floor).

Sharding: core c owns output rows [16c,16c+16) for rowpart and cols
[16c,16c+16) for colpart -- 2 families x 4 quarters x 4 tile-quads = 32
instances of B matmuls; 16 slab groups (fam x quarter x halfblock) all
resident in SBUF, loaded once.  Outputs evict as bf16; the full result is
host-assembled as out[y,x] = rowpart_{y//16}[y%16, x] + colpart_{x//16}
[x%16, y].  The SPMD instruction stream is identical on every core.
"""

from contextlib import ExitStack

import ml_dtypes
import numpy as np

import concourse.bass as bass
from concourse import mybir
from concourse.bass_utils import run_bass_kernel_spmd

BF16 = ml_dtypes.bfloat16
E3 = ml_dtypes.float8_e3m4

# Problem constants (hardcoded per the harness contract).
N, C, A, R = 4, 128, 180, 184
H = W = 128
NC = N * C  # 512
NCORES = 8
TY, TX = 4, 32  # tile shape (iterate-dim extent x gather-dim extent)
NQ = 128 // TX  # quarters
NG = 16 // TY  # tile quads per (core, quarter)
GH = 8  # halfblock rows (slab window granularity along the iterate dim)
NGRP = 2 * NQ * 2  # slab groups: fam x quarter x halfblock
NBANK = 8  # PSUM banks
NWRING = 4  # weight ring depth (instances)
NOBUF = 8  # output staging buffers

_cache = {}


def _rho_table():
    """r[k, y, x] int32 rho index; always in [0, R) for this geometry."""
    if "r" not in _cache:
        k = np.arange(A)
        theta = k * (np.pi / A)
        cos_t, sin_t = np.cos(theta), np.sin(theta)
        y, x = np.meshgrid(np.arange(H), np.arange(W), indexing="ij")
        xc = (x - W // 2).astype(np.float64)
        yc = (y - H // 2).astype(np.float64)
        r = np.round(cos_t[:, None, None] * xc[None] + sin_t[:, None, None] * yc[None])
        r = r.astype(np.int64) + R // 2
        assert (r >= 0).all() and (r < R).all()
        _cache["r"] = r.astype(np.int32)
        _cache["fam_row"] = np.abs(sin_t) >= np.abs(cos_t)
    return _cache["r"]


def _blk(r, f, c, q, h, k):
    """The (halfblock x quarter) index block for angle k, family f."""
    s = 16 * c + GH * h
    if f == 0:
        return r[k, s : s + GH, TX * q : TX * q + TX]
    return r[k, TX * q : TX * q + TX, s : s + GH]


def _geometry():
    """Families, global lane widths, FFD bin layout (SPMD-uniform)."""
    if "geo" in _cache:
        return _cache["geo"]
    r = _rho_table()
    fam_row = _cache["fam_row"]

    width = {}
    for k in range(A):
        f = 0 if fam_row[k] else 1
        ws = [
            int(_blk(r, f, c, q, h, k).max() - _blk(r, f, c, q, h, k).min()) + 1
            for c in range(NCORES)
            for q in range(NQ)
            for h in range(2)
        ]
        width[k] = max(ws)
        assert width[k] <= 128

    def ffd(items):
        bins = []
        for w_, kk in sorted(items, reverse=True):
            for b in bins:
                if b[0] + w_ <= 128:
                    b[0] += w_
                    b[1].append((kk, w_))
                    break
            else:
                bins.append([w_, [(kk, w_)]])
        return [b[1] for b in bins]

    fams = []
    for f in range(2):
        ks = [k for k in range(A) if fam_row[k] == (f == 0)]
        fams.append(ffd([(width[k], k) for k in ks]))
    B = max(len(fams[0]), len(fams[1]))
    bins = [[], []]
    for f in range(2):
        for lane_list in fams[f]:
            out, base = [], 0
            for k, w_ in lane_list:
                out.append((k, w_, base))
                base += w_
            bins[f].append(out)
        while len(bins[f]) < B:
            bins[f].append([])

    _cache["geo"] = dict(bins=bins, B=B)
    return _cache["geo"]


def _host_weights():
    """Per-core one-hot tables [32, 128, B*128] e3m4 and slab offsets."""
    if "wts" in _cache:
        return _cache["wts"], _cache["offs"]
    geo = _geometry()
    r = _rho_table()
    B = geo["B"]
    bins = geo["bins"]

    wts = []
    offs = []  # offs[c][(f,q,h,bi)] = [(k, w, base, o)]
    for c in range(NCORES):
        w_tab = np.zeros((32, 128, B * 128), E3)
        omap = {}
        for f in range(2):
            for q in range(NQ):
                for h in range(2):
                    for bi, lanes in enumerate(bins[f]):
                        entry = []
                        for k, wd, base in lanes:
                            blk = _blk(r, f, c, q, h, k)
                            lo, hi = int(blk.min()), int(blk.max())
                            o = min(lo, R - wd)
                            assert 0 <= o and o + wd > hi
                            entry.append((k, wd, base, o))
                        omap[(f, q, h, bi)] = entry
                for g in range(NG):
                    i = f * 16 + q * NG + g
                    h = g // 2
                    for bi in range(B):
                        for k, wd, base, o in omap[(f, q, h, bi)]:
                            blk = _blk(r, f, c, q, h, k)
                            loc = TY * (g % 2)
                            # sub[it, gt]: iterate-dim-major tile indices
                            sub = (
                                blk[loc : loc + TY, :]
                                if f == 0
                                else blk[:, loc : loc + TY].T
                            )
                            rows = (sub - o + base).ravel()
                            w_tab[i, rows, bi * 128 + np.arange(128)] = 1
        wts.append(w_tab)
        offs.append(omap)
    _cache["wts"] = wts
    _cache["offs"] = offs
    return wts, offs


def _build_nc():
    if "nc" in _cache:
        return _cache["nc"]
    geo = _geometry()
    B = geo["B"]

    nc = bass.Bass("TRN2", debug=False, target_bir_lowering=False, num_devices=NCORES)
    wts_d = nc.dram_tensor(
        "wts", [32, 128, B * 128], mybir.dt.float8e3, kind="ExternalInput"
    ).ap()
    slab_d = nc.dram_tensor(
        "slabs", [NGRP, 128, B * NC], mybir.dt.float8e3, kind="ExternalInput"
    ).ap()
    outr_d = nc.dram_tensor(
        "outr", [16, 128, NC], mybir.dt.bfloat16, kind="ExternalOutput"
    ).ap()
    outc_d = nc.dram_tensor(
        "outc", [16, 128, NC], mybir.dt.bfloat16, kind="ExternalOutput"
    ).ap()

    ctx = ExitStack()
    _cache["ctx"] = ctx
    slabs_sb = ctx.enter_context(
        nc.sbuf_tensor("slabs_sb", [128, NGRP * B * NC], mybir.dt.float8e3)
    )
    wring = ctx.enter_context(
        nc.sbuf_tensor("wring", [128, NWRING * B * 128], mybir.dt.float8e3)
    )
    obuf = ctx.enter_context(
        nc.sbuf_tensor("obuf", [128, NOBUF * NC], mybir.dt.bfloat16)
    )
    ps = [
        ctx.enter_context(nc.psum_tensor(f"ps{i}", [128, NC], mybir.dt.float32))
        for i in range(NBANK)
    ]
    mm_sem = ctx.enter_context(nc.semaphore("mm_sem"))
    cp_sem = ctx.enter_context(nc.semaphore("cp_sem"))
    dump_sems = [
        ctx.enter_context(nc.semaphore(f"dump{s}")) for s in range(NOBUF)
    ]
    wt_sems = [ctx.enter_context(nc.semaphore(f"wt{s}")) for s in range(NWRING)]
    sl_sems = [ctx.enter_context(nc.semaphore(f"sl{g}")) for g in range(NGRP)]
    block = ctx.enter_context(nc.Block(no_gpsimd_drain=True))

    def _slab_dma(eng, grp):
        col = grp * B * NC
        eng.dma_start(
            slabs_sb[:, col : col + B * NC], slab_d[grp]
        ).then_inc(sl_sems[grp], 16)

    @block.gpsimd
    def _(gpsimd):
        for i in range(32):
            if i >= NWRING:
                gpsimd.wait_ge(mm_sem, i - NWRING + 1)
            base = (i % NWRING) * B * 128
            gpsimd.dma_start(
                wring[:, base : base + B * 128], wts_d[i]
            ).then_inc(wt_sems[i % NWRING], 16)

    @block.scalar
    def _(scalar):
        # family-0 slab groups stream up front in consumption order;
        # family-1 groups are issued once the pipeline is running.
        for grp in range(8):
            _slab_dma(scalar, grp)
        for i in range(32):
            scalar.wait_ge(mm_sem, i + 1)
            if i >= NOBUF:
                scalar.wait_ge(dump_sems[i % NOBUF], 16 * TY * (i // NOBUF))
            col = (i % NOBUF) * NC
            scalar.copy(obuf[:, col : col + NC], ps[i % NBANK][:]).then_inc(cp_sem, 1)
            if i == 2:
                for grp in range(8, 12):
                    _slab_dma(scalar, grp)
            if i == 8:
                for grp in range(12, 16):
                    _slab_dma(scalar, grp)

    @block.tensor
    def _(tensor):
        # Warm the PE clock gate during the DMA prologue with junk matmuls
        # (bank 7 is cleared by instance 7's start=True).
        wq = (NWRING - 1) * B * 128
        for _ in range(48):
            tensor.matmul(
                out=ps[NBANK - 1][:, :128],
                lhsT=wring[:, wq : wq + 128],
                rhs=wring[:, wq : wq + 128],
                start=True,
                stop=True,
                skip_group_check=True,
            )
        for i in range(32):
            f, rem = i // 16, i % 16
            q, g = rem // NG, rem % NG
            grp = f * 8 + q * 2 + g // 2
            if i >= NBANK:
                tensor.wait_ge(cp_sem, i - NBANK + 1)
            tensor.wait_ge(wt_sems[i % NWRING], 16 * (i // NWRING + 1))
            if g % 2 == 0:
                tensor.wait_ge(sl_sems[grp], 16)
            wbase = (i % NWRING) * B * 128
            scol = grp * B * NC
            for b in range(B):
                mm = tensor.matmul(
                    out=ps[i % NBANK][:],
                    lhsT=wring[:, wbase + b * 128 : wbase + (b + 1) * 128],
                    rhs=slabs_sb[:, scol + b * NC : scol + (b + 1) * NC],
                    start=(b == 0),
                    stop=(b == B - 1),
                )
            mm.then_inc(mm_sem, 1)

    @block.sync
    def _(sync):
        for i in range(32):
            sync.wait_ge(cp_sem, i + 1)
            f, rem = i // 16, i % 16
            q, g = rem // NG, rem % NG
            out_d = outr_d if f == 0 else outc_d
            col = (i % NOBUF) * NC
            for t in range(TY):
                sync.dma_start(
                    out_d[TY * g + t, TX * q : TX * q + TX],
                    obuf[t * TX : (t + 1) * TX, col : col + NC],
                ).then_inc(dump_sems[i % NOBUF], 16)

    _cache["nc"] = nc
    return nc


def _install_ntff_hook():
    """Provide the antenv.axon_hooks shim the image lacks, wiring the
    ctypes NTFF profiler from trn_agent_boot."""
    import sys
    import types

    if "antenv.axon_hooks" in sys.modules:
        return
    import antenv
    from trn_agent_boot.trn_boot import _ntff_profile_via_ctypes

    mod = types.ModuleType("antenv.axon_hooks")
    hook = _ntff_profile_via_ctypes("/opt/axon/libaxon_pjrt.so")
    mod.get_axon_ntff_profile_hook = lambda: hook
    mod.set_axon_ntff_profile_hook = lambda h: None
    sys.modules["antenv.axon_hooks"] = mod
    antenv.axon_hooks = mod


def hw_exec_time_ns(trace_cores=None):
    """Re-run the last kernel() invocation with tracing; return max core ns."""
    _install_ntff_hook()
    nc = _cache["nc"]
    res = run_bass_kernel_spmd(
        nc,
        _cache["in_maps"],
        core_ids=list(range(NCORES)),
        trace=True,
        trace_cores=trace_cores,
    )
    _cache["trace"] = res
    return res.exec_time_ns


def kernel(accumulator, out_H=128, out_W=128, numangle=180, numrho=184):
    accumulator = np.asarray(accumulator, np.float32)
    assert accumulator.shape == (N, C, A, R), accumulator.shape
    assert int(out_H) == H and int(out_W) == W
    assert int(numangle) == A and int(numrho) == R

    geo = _geometry()
    B = geo["B"]
    wts, offs = _host_weights()
    nc = _build_nc()

    # acc_t[k, rho, nc] e3m4 - slab source (single rounding from f32).
    acc_t = np.ascontiguousarray(
        accumulator.reshape(NC, A, R).transpose(1, 2, 0)
    ).astype(E3)

    in_maps = []
    for c in range(NCORES):
        slabs = np.zeros((NGRP, 128, B, NC), E3)
        for f in range(2):
            for q in range(NQ):
                for h in range(2):
                    grp = f * 8 + q * 2 + h
                    for bi in range(B):
                        for k, wd, base, o in offs[c][(f, q, h, bi)]:
                            slabs[grp, base : base + wd, bi] = acc_t[k, o : o + wd]
        in_maps.append({"wts": wts[c], "slabs": slabs.reshape(NGRP, 128, B * NC)})
    _cache["in_maps"] = in_maps
    res = run_bass_kernel_spmd(nc, in_maps, core_ids=list(range(NCORES)))

    # Unshard: out[y, x, nc] = rowpart[y] + colpart[x] (transposed).
    total = np.zeros((H, W, NC), np.float64)
    for c in range(NCORES):
        total[16 * c : 16 * c + 16] += res.results[c]["outr"].astype(np.float64)
        total[:, 16 * c : 16 * c + 16] += (
            res.results[c]["outc"].astype(np.float64).transpose(1, 0, 2)
        )
    return total.transpose(2, 0, 1).reshape(N, C, H, W).astype(np.float32)
